# revision 11
# baseline (speedup 1.0000x reference)
"""HBMP (3-branch LSTM + BiLSTM + global max pool) Trainium2 kernel.

Model (B=64, T=512, E=300, H=512, NB=3 branches):
  per branch: h1 = LSTM(x); hf = LSTM(h1); hb = rev(LSTM(rev(h1)))
  emb = maxpool_T(concat([hf, hb], -1));  out = concat over branches [B, 3*2H]

Mapping onto 6 NeuronCores (task-parallel; the recurrent matmul cost is
weight-streaming-bound and independent of batch, so batch stays whole):
  core c handles (branch = c%3, direction = fwd if c<3 else bwd):
    loop1 (For_i, 4 steps/iter): uni LSTM scan with the x-projection
      fused into each step's PSUM accumulation (bias folded into x's
      E-padding as a ones-row); the transposed h stream goes to DRAM bf16.
    loop2 (For_i, 4 steps/iter): dir LSTM scan with the h1-projection
      fused into each step's PSUM accumulation, consuming the hT stream
      (reversed addressing for bwd cores via partition_id register
      arithmetic - one SPMD program serves both directions) + running max.
Host gathers the 6 rmax outputs into [64, 3072].

Step layout ("dup-batch"): gate pre-activations live as two PSUM bank
tiles zb[n] [128, 512] with batch duplicated across partition halves
(lane b+64j holds hidden slice j*256..(j+1)*256); matmuls write the
j=1 half via tile_position=(0,64) column offset with the same [128,64]
stationary (state / x / h1 chunk), so every ScalarE/VectorE op runs on
all 128 lanes with half the per-lane elements.  Bank 0 = gates f|i,
bank 1 = o|g, each closing its own accumulation group so the f|i
sigmoid overlaps the o|g matmuls.  h is re-transposed on PE (4x 64x128
blocks from partition bases 0/64).
"""
import sys

sys.path.insert(0, "/opt/trn_rl_repo")

import numpy as np
import ml_dtypes

BF16 = ml_dtypes.bfloat16
B, T, E, H = 64, 512, 300, 512
FOUR_H = 4 * H
HB = 2 * H  # per-j-half moving width (1024)
NB = 3
UNROLL = 4

_CACHE = {}


def _build_program(rep=1, with_bias=True):
    import concourse.bass as bass
    import concourse.tile as tile
    from concourse import bacc, mybir

    F32 = mybir.dt.float32
    BF = mybir.dt.bfloat16
    Sig = mybir.ActivationFunctionType.Sigmoid
    Tanh = mybir.ActivationFunctionType.Tanh
    ds = bass.ds
    Q = H // 2  # 256: per-gate per-j-half column count

    nc = bacc.Bacc("TRN2", target_bir_lowering=False, debug=False,
                   enable_asserts=False, num_devices=6)

    d = {}
    # x transposed + padded: xT[t,p,k,b] = xpad[b,t,k*128+p]; row 300 == 1.0
    d["xT"] = nc.dram_tensor("xT", [T, 128, 3, B], BF, kind="ExternalInput").ap()
    # weights reordered for dup-batch: w[p, k, j, q*Q + c]
    #   = W[k*128+p, gate(q)*H//... see _prep_core]  (gate order f,i,o,g)
    d["wxu"] = nc.dram_tensor("wxu", [128, 3, 2, HB], BF, kind="ExternalInput").ap()
    d["whu"] = nc.dram_tensor("whu", [128, 4, 2, HB], BF, kind="ExternalInput").ap()
    d["wxd"] = nc.dram_tensor("wxd", [128, 4, 2, HB], BF, kind="ExternalInput").ap()
    d["whd"] = nc.dram_tensor("whd", [128, 4, 2, HB], BF, kind="ExternalInput").ap()
    d["bdT"] = nc.dram_tensor("bdT", [1, 2, HB], BF, kind="ExternalInput").ap()
    d["one1"] = nc.dram_tensor("one1", [1, B], BF, kind="ExternalInput").ap()
    d["id64"] = nc.dram_tensor("id64", [B, B], F32, kind="ExternalInput").ap()
    d["hT"] = nc.dram_tensor("hT", [T, 128, 4 * B], BF, kind="Internal").ap()
    d["rmax"] = nc.dram_tensor("rmax", [B, H], F32, kind="ExternalOutput").ap()

    def build(tc):
        with (
            tc.tile_pool(name="w", bufs=1) as wp,
            tc.tile_pool(name="state", bufs=1) as sp,
            tc.tile_pool(name="io", bufs=2) as iop,
            tc.tile_pool(name="g", bufs=2) as gp,
            tc.tile_pool(name="zps", bufs=2, space="PSUM") as zp,
            tc.tile_pool(name="tps", bufs=2, space="PSUM") as tp,
        ):
            # --- persistent weights ---
            wxu = wp.tile([128, 3, 2, HB], BF, tag="wxu")
            nc.sync.dma_start(wxu[:], d["wxu"])
            whu = wp.tile([128, 4, 2, HB], BF, tag="whu")
            nc.sync.dma_start(whu[:], d["whu"])
            wxd = wp.tile([128, 4, 2, HB], BF, tag="wxd")
            nc.sync.dma_start(wxd[:], d["wxd"])
            whd = wp.tile([128, 4, 2, HB], BF, tag="whd")
            nc.sync.dma_start(whd[:], d["whd"])
            bdT = wp.tile([1, 2, HB], BF, tag="bdT")
            nc.sync.dma_start(bdT[:], d["bdT"])
            one1 = wp.tile([1, B], BF, tag="one1")
            nc.sync.dma_start(one1[:], d["one1"])
            id64 = wp.tile([B, B], F32, tag="id64")
            nc.sync.dma_start(id64[:], d["id64"])

            # reversal selector: 0 for cores 0-2 (fwd), 1 for cores 3-5 (bwd)
            s = nc.sync.partition_id() >= 3

            # --- per-scan state ---
            hTs = sp.tile([128, 4, B], BF, tag="hTs")       # transposed h state
            st = sp.tile([128, 2 * Q], F32, tag="st")       # [c | tanh g]
            rmax = sp.tile([B, H], F32, tag="rmax")

            def step(xk_stat, nk, wx_sb, wh_sb, delta, bias, hstore, do_rmax):
                """One LSTM step in dup-batch layout.
                xk_stat(k) -> bf16 stationary [128, B] for the input-
                projection k-chunk (x_t or h1_t); wx/wh moving [128,nk,2,HB].
                Bank n=0 holds gates f|i, n=1 holds o|g (Q cols each)."""
                zb = [zp.tile([128, 512], F32, tag=f"zb{n}", name=f"zb{n}")
                      for n in range(2)]
                # Each (bank, j-half) quadrant is its own PSUM accumulation
                # group (the has_written clear covers the full bank width but
                # only the addressed partitions).  j=0/j=1 pairs are issued
                # back-to-back: they hit disjoint PE column groups, so their
                # moving streams overlap on different sub-arrays.
                # input-projection (+bias) matmuls: independent of the
                # recurrent state, scheduled into the previous step's gaps
                TP = (None, (0, 64))
                for n in range(2):
                    for k in range(nk):
                        for j in range(2):
                            nc.tensor.matmul(
                                zb[n][64 * j:64 * j + 64, :], xk_stat(k),
                                wx_sb[:, k, j, bass.ts(n, 512)],
                                start=(k == 0), stop=False, tile_position=TP[j])
                    if bias is not None:
                        for j in range(2):
                            nc.tensor.matmul(
                                zb[n][64 * j:64 * j + 64, :], one1[:],
                                bias[:, j, bass.ts(n, 512)],
                                start=False, stop=False, tile_position=TP[j])
                # recurrent matmuls; bank n closes its own groups so the
                # f|i sigmoid overlaps bank 1's matmuls
                for n in range(2):
                    for k in range(4):
                        for j in range(2):
                            nc.tensor.matmul(
                                zb[n][64 * j:64 * j + 64, :], hTs[:, k, :],
                                wh_sb[:, k, j, bass.ts(n, 512)],
                                start=False, stop=(k == 3), tile_position=TP[j])
                # gates: zb0 = [f|i], zb1 = [o|g]
                ga = gp.tile([128, 2 * Q], F32, tag="ga")   # [sf | si]
                go = gp.tile([128, Q], F32, tag="go")       # so
                nc.scalar.activation(ga[:], zb[0][:], Sig)
                nc.scalar.activation(st[:, Q:2 * Q], zb[1][:, Q:2 * Q], Tanh)
                nc.scalar.activation(go[:], zb[1][:, 0:Q], Sig)
                t12 = gp.tile([128, 2 * Q], F32, tag="t12")
                nc.vector.tensor_mul(t12[:], ga[:], st[:])
                nc.vector.tensor_add(st[:, 0:Q], t12[:, 0:Q], t12[:, Q:2 * Q])
                tc_t = gp.tile([128, Q], F32, tag="tc")
                nc.scalar.activation(tc_t[:], st[:, 0:Q], Tanh)
                # h in flat [64, 512] layout via two cross-base muls
                # (hf[b, j*256+c] = h[b, hid]); transposes then run from
                # partition base 0 (base-64 transposes hang the device)
                hf = gp.tile([B, H], F32, tag="hf")
                for j in range(2):
                    nc.vector.tensor_mul(hf[:, bass.ts(j, Q)],
                                         go[64 * j:64 * j + 64, :],
                                         tc_t[64 * j:64 * j + 64, :])
                if do_rmax:
                    nc.vector.tensor_max(rmax[:], rmax[:], hf[:])
                pT = tp.tile([128, 4, B], F32, tag="pT")
                for k in range(4):
                    nc.tensor.transpose(pT[:, k, :], hf[:, bass.ts(k, 128)],
                                        id64[:])
                nc.vector.tensor_copy(hTs[:], pT[:])
                if hstore is not None:
                    nc.vector.tensor_copy(hstore[:, delta, :],
                                          pT[:].rearrange("p k b -> p (k b)"))

            # ================= loop 1: uni scan =================
            nc.vector.memset(hTs[:], 0.0)
            nc.vector.memset(st[:], 0.0)
            with tc.For_i(0, T, UNROLL, staggered_reset=True) as i:
                xt = iop.tile([128, UNROLL, 3, B], BF, tag="xt")
                nc.sync.dma_start(
                    xt[:], d["xT"][ds(i, UNROLL)].rearrange("t p k b -> p t k b"))
                hst = iop.tile([128, UNROLL, 4 * B], BF, tag="hst")
                for dt in range(UNROLL):
                    if dt:
                        tc.stage_boundary()
                    step(lambda k, dt=dt: xt[:, dt, k, :], 3, wxu, whu,
                         dt, None, hst, False)
                nc.sync.dma_start(
                    d["hT"][ds(i, UNROLL)].rearrange("t p e -> p t e"), hst[:])

            # ================= loop 2: dir scan =================
            nc.vector.memset(hTs[:], 0.0)
            nc.vector.memset(st[:], 0.0)
            nc.vector.memset(rmax[:], -1e30)
            with tc.For_i(0, T, UNROLL, staggered_reset=True) as i:
                ht1 = iop.tile([128, UNROLL, 4, B], BF, tag="ht1")
                for dt in range(UNROLL):
                    # fwd: t = i+dt ; bwd: t = (T-1) - (i+dt)
                    tt = i + dt
                    tsrc = nc.s_assert_within(
                        tt + s * (T - 1 - 2 * tt), 0, T - 1,
                        skip_runtime_assert=True)
                    nc.sync.dma_start(
                        ht1[:, dt, :, :].rearrange("p k b -> p (k b)"),
                        d["hT"][ds(tsrc, 1)].rearrange("t p e -> (t p) e"))
                for dt in range(UNROLL):
                    if dt:
                        tc.stage_boundary()
                    step(lambda k, dt=dt: ht1[:, dt, k, :], 4, wxd, whd,
                         dt, bdT if with_bias else None, None, True)
            nc.sync.dma_start(d["rmax"], rmax[:])

    with tile.TileContext(nc) as tc:
        for _ in range(rep):
            build(tc)
    nc.compile()
    return nc


_GATE_PERM = np.r_[H:2 * H, 0:H, 3 * H:4 * H, 2 * H:3 * H]  # [i f g o]->[f i o g]


def _dup_cols(w):
    """[rows, 4H] gate-ordered [f i o g] -> [rows, 2j, (n,q',c)=HB] so that
    moving slice w2[:, j, n*512:(n+1)*512] covers gates (2n, 2n+1), j-half."""
    rows = w.shape[0]
    # w[r, q*H + j*256 + c] -> w2[r, j, q*256 + c]
    w5 = w.reshape(rows, 4, 2, 256)            # [r, q, j, c]
    return np.ascontiguousarray(w5.transpose(0, 2, 1, 3).reshape(rows, 2, HB))


def _prep_shared(x):
    """x [B,T,E] -> xT [T,128,3,64] bf16 with ones-row at E-index 300."""
    xpad = np.zeros((B, T, 384), np.float32)
    xpad[:, :, :E] = x
    xpad[:, :, E] = 1.0
    xT = xpad.transpose(1, 2, 0).reshape(T, 3, 128, B).transpose(0, 2, 1, 3)
    return np.ascontiguousarray(xT.astype(BF16))


def _chunk(w, nk):
    """[nk*128, 2, HB] -> [128, nk, 2, HB] bf16."""
    return np.ascontiguousarray(
        w.reshape(nk, 128, 2, HB).transpose(1, 0, 2, 3).astype(BF16))


def _prep_core(xT, wx_u, wh_u, b_u, wx_d, wh_d, b_d):
    wx_u = np.asarray(wx_u, np.float32)[:, _GATE_PERM]
    wh_u = np.asarray(wh_u, np.float32)[:, _GATE_PERM]
    b_u = np.asarray(b_u, np.float32)[_GATE_PERM]
    wx_d = np.asarray(wx_d, np.float32)[:, _GATE_PERM]
    wh_d = np.asarray(wh_d, np.float32)[:, _GATE_PERM]
    b_d = np.asarray(b_d, np.float32)[_GATE_PERM]
    wxu_pad = np.zeros((384, FOUR_H), np.float32)
    wxu_pad[:E] = wx_u
    wxu_pad[E] = b_u
    return {
        "xT": xT,
        "wxu": _chunk(_dup_cols(wxu_pad), 3),
        "whu": _chunk(_dup_cols(wh_u), 4),
        "wxd": _chunk(_dup_cols(wx_d), 4),
        "whd": _chunk(_dup_cols(wh_d), 4),
        "bdT": np.ascontiguousarray(_dup_cols(b_d[None, :]).astype(BF16)),
        "one1": np.ones((1, B), BF16),
        "id64": np.eye(B, dtype=np.float32),
    }


def _run(in_maps, rep=1, with_bias=True):
    from concourse.bass_utils import run_bass_kernel_spmd
    key = f"nc{rep}_{with_bias}"
    if key not in _CACHE:
        _CACHE[key] = _build_program(rep, with_bias)
    return run_bass_kernel_spmd(_CACHE[key], in_maps, core_ids=list(range(6)))


def build_in_maps(x, uni_Wx, uni_Wh, uni_b, fwd_Wx, fwd_Wh, fwd_b,
                  bwd_Wx, bwd_Wh, bwd_b):
    xT = _prep_shared(np.asarray(x, np.float32))
    in_maps = []
    for c in range(6):
        br = c % 3
        if c < 3:
            wx_d, wh_d, b_d = fwd_Wx[br], fwd_Wh[br], fwd_b[br]
        else:
            wx_d, wh_d, b_d = bwd_Wx[br], bwd_Wh[br], bwd_b[br]
        in_maps.append(_prep_core(xT, uni_Wx[br], uni_Wh[br], uni_b[br],
                                  wx_d, wh_d, b_d))
    return in_maps


def kernel(x, uni_Wx, uni_Wh, uni_b, fwd_Wx, fwd_Wh, fwd_b,
           bwd_Wx, bwd_Wh, bwd_b):
    in_maps = build_in_maps(x, uni_Wx, uni_Wh, uni_b, fwd_Wx, fwd_Wh, fwd_b,
                            bwd_Wx, bwd_Wh, bwd_b)
    wb = bool(np.any(np.asarray(fwd_b)) or np.any(np.asarray(bwd_b)))
    res = _run(in_maps, with_bias=wb)
    out = np.empty((B, NB * 2 * H), np.float32)
    for c in range(6):
        br = c % 3
        off = br * 2 * H + (0 if c < 3 else H)
        out[:, off:off + H] = res.results[c]["rmax"]
    return out


# revision 13
# speedup vs baseline: 1.3797x; 1.3797x over previous
"""HBMP (3-branch LSTM + BiLSTM + global max pool) Trainium2 kernel.

Model (B=64, T=512, E=300, H=512, NB=3 branches):
  per branch: h1 = LSTM(x); hf = LSTM(h1); hb = rev(LSTM(rev(h1)))
  emb = maxpool_T(concat([hf, hb], -1));  out = concat over branches [B, 3*2H]

Mapping onto 6 NeuronCores (task-parallel; the recurrent matmul cost is
weight-streaming-bound and independent of batch, so batch stays whole):
  core c handles (branch = c%3, direction = fwd if c<3 else bwd):
    loop1 (For_i, 4 steps/iter): uni LSTM scan with the x-projection
      fused into each step's PSUM accumulation (bias folded into x's
      E-padding as a ones-row); the transposed h stream goes to DRAM bf16.
    loop2 (For_i, 4 steps/iter): dir LSTM scan with the h1-projection
      fused into each step's PSUM accumulation, consuming the hT stream
      (reversed addressing for bwd cores via partition_id register
      arithmetic - one SPMD program serves both directions) + running max.
Host gathers the 6 rmax outputs into [64, 3072].

Step layout ("dup-batch"): gate pre-activations live as two PSUM bank
tiles zb[n] [128, 512] with batch duplicated across partition halves
(lane b+64j holds hidden slice j*256..(j+1)*256); matmuls write the
j=1 half via tile_position=(0,64) column offset with the same [128,64]
stationary (state / x / h1 chunk), so every ScalarE/VectorE op runs on
all 128 lanes with half the per-lane elements.  Bank 0 = gates f|i,
bank 1 = o|g, each closing its own accumulation group so the f|i
sigmoid overlaps the o|g matmuls.  h is re-transposed on PE (4x 64x128
blocks from partition bases 0/64).
"""
import sys

sys.path.insert(0, "/opt/trn_rl_repo")

import numpy as np
import ml_dtypes

BF16 = ml_dtypes.bfloat16
B, T, E, H = 64, 512, 300, 512
FOUR_H = 4 * H
HB = 2 * H  # per-j-half moving width (1024)
NB = 3
UNROLL = 8

_CACHE = {}


def _build_program(rep=1, with_bias=True):
    import concourse.bass as bass
    import concourse.tile as tile
    from concourse import bacc, mybir

    F32 = mybir.dt.float32
    BF = mybir.dt.bfloat16
    Sig = mybir.ActivationFunctionType.Sigmoid
    Tanh = mybir.ActivationFunctionType.Tanh
    ds = bass.ds
    Q = H // 2  # 256: per-gate per-j-half column count

    nc = bacc.Bacc("TRN2", target_bir_lowering=False, debug=False,
                   enable_asserts=False, num_devices=6)

    d = {}
    # x transposed + padded: xT[t,p,k,b] = xpad[b,t,k*128+p]; row 300 == 1.0
    d["xT"] = nc.dram_tensor("xT", [T, 128, 3, B], BF, kind="ExternalInput").ap()
    # weights reordered for dup-batch: w[p, k, j, q*Q + c]
    #   = W[k*128+p, gate(q)*H//... see _prep_core]  (gate order f,i,o,g)
    d["wxu"] = nc.dram_tensor("wxu", [128, 3, 2, HB], BF, kind="ExternalInput").ap()
    d["whu"] = nc.dram_tensor("whu", [128, 4, 2, HB], BF, kind="ExternalInput").ap()
    d["wxd"] = nc.dram_tensor("wxd", [128, 4, 2, HB], BF, kind="ExternalInput").ap()
    d["whd"] = nc.dram_tensor("whd", [128, 4, 2, HB], BF, kind="ExternalInput").ap()
    d["bdT"] = nc.dram_tensor("bdT", [1, 2, HB], BF, kind="ExternalInput").ap()
    d["one1"] = nc.dram_tensor("one1", [1, B], BF, kind="ExternalInput").ap()
    d["id64"] = nc.dram_tensor("id64", [B, B], F32, kind="ExternalInput").ap()
    d["hT"] = nc.dram_tensor("hT", [T, 128, 4 * B], BF, kind="Internal").ap()
    d["rmax"] = nc.dram_tensor("rmax", [B, H], F32, kind="ExternalOutput").ap()

    def build(tc):
        with (
            tc.tile_pool(name="w", bufs=1) as wp,
            tc.tile_pool(name="state", bufs=1) as sp,
            tc.tile_pool(name="io", bufs=2) as iop,
            tc.tile_pool(name="g", bufs=2) as gp,
            tc.tile_pool(name="zps", bufs=2, space="PSUM") as zp,
            tc.tile_pool(name="tps", bufs=2, space="PSUM") as tp,
        ):
            # --- persistent weights ---
            wxu = wp.tile([128, 3, 2, HB], BF, tag="wxu")
            nc.sync.dma_start(wxu[:], d["wxu"])
            whu = wp.tile([128, 4, 2, HB], BF, tag="whu")
            nc.sync.dma_start(whu[:], d["whu"])
            wxd = wp.tile([128, 4, 2, HB], BF, tag="wxd")
            nc.sync.dma_start(wxd[:], d["wxd"])
            whd = wp.tile([128, 4, 2, HB], BF, tag="whd")
            nc.sync.dma_start(whd[:], d["whd"])
            bdT = wp.tile([1, 2, HB], BF, tag="bdT")
            nc.sync.dma_start(bdT[:], d["bdT"])
            one1 = wp.tile([1, B], BF, tag="one1")
            nc.sync.dma_start(one1[:], d["one1"])
            id64 = wp.tile([B, B], F32, tag="id64")
            nc.sync.dma_start(id64[:], d["id64"])

            # reversal selector: 0 for cores 0-2 (fwd), 1 for cores 3-5 (bwd)
            s = nc.sync.partition_id() >= 3
            HINTS = (mybir.EngineType.PE,)

            # --- per-scan state ---
            hTs = sp.tile([128, 4, B], BF, tag="hTs")       # transposed h state
            st = sp.tile([128, 2 * Q], F32, tag="st")       # [c | tanh g]
            rmax = sp.tile([B, H], F32, tag="rmax")

            TP = (None, (0, 64))

            def emit_proj(xk_stat, nk, wx_sb, bias):
                """Input-projection (+bias) matmuls for one step: independent
                of the recurrent state, emitted one step early so the PE runs
                them inside the previous step's gate window.  Each (bank,
                j-half) quadrant is its own PSUM accumulation group (the
                has_written clear covers the full bank width but only the
                addressed partitions)."""
                zb = [zp.tile([128, 512], F32, tag=f"zb{n}", name=f"zb{n}")
                      for n in range(2)]
                for n in range(2):
                    for k in range(nk):
                        for j in range(2):
                            nc.tensor.matmul(
                                zb[n][64 * j:64 * j + 64, :], xk_stat(k),
                                wx_sb[:, k, j, bass.ts(n, 512)],
                                start=(k == 0), stop=False, tile_position=TP[j])
                    if bias is not None:
                        for j in range(2):
                            nc.tensor.matmul(
                                zb[n][64 * j:64 * j + 64, :], one1[:],
                                bias[:, j, bass.ts(n, 512)],
                                start=False, stop=False, tile_position=TP[j])
                return zb

            def emit_rec(zb, wh_sb):
                # recurrent matmuls; bank n closes its own groups so the
                # f|i sigmoid overlaps bank 1's matmuls
                for n in range(2):
                    for k in range(4):
                        for j in range(2):
                            nc.tensor.matmul(
                                zb[n][64 * j:64 * j + 64, :], hTs[:, k, :],
                                wh_sb[:, k, j, bass.ts(n, 512)],
                                start=False, stop=(k == 3), tile_position=TP[j])

            def step_tail(zb, delta, hstore, do_rmax):
                # gates: zb0 = [f|i], zb1 = [o|g]
                ga = gp.tile([128, 2 * Q], F32, tag="ga")   # [sf | si]
                go = gp.tile([128, Q], F32, tag="go")       # so
                nc.scalar.activation(ga[:], zb[0][:], Sig)
                nc.scalar.activation(st[:, Q:2 * Q], zb[1][:, Q:2 * Q], Tanh)
                nc.scalar.activation(go[:], zb[1][:, 0:Q], Sig)
                t12 = gp.tile([128, 2 * Q], F32, tag="t12")
                nc.vector.tensor_mul(t12[:], ga[:], st[:])
                nc.vector.tensor_add(st[:, 0:Q], t12[:, 0:Q], t12[:, Q:2 * Q])
                tc_t = gp.tile([128, Q], F32, tag="tc")
                nc.scalar.activation(tc_t[:], st[:, 0:Q], Tanh)
                # h in flat [64, 512] layout via two cross-base muls
                # (hf[b, j*256+c] = h[b, hid]); transposes then run from
                # partition base 0 (base-64 transposes hang the device)
                hf = gp.tile([B, H], F32, tag="hf")
                for j in range(2):
                    nc.vector.tensor_mul(hf[:, bass.ts(j, Q)],
                                         go[64 * j:64 * j + 64, :],
                                         tc_t[64 * j:64 * j + 64, :])
                if do_rmax:
                    nc.vector.tensor_max(rmax[:], rmax[:], hf[:])
                pT = tp.tile([128, 4, B], F32, tag="pT")
                for k in range(4):
                    nc.tensor.transpose(pT[:, k, :], hf[:, bass.ts(k, 128)],
                                        id64[:])
                nc.vector.tensor_copy(hTs[:], pT[:])
                if hstore is not None:
                    nc.vector.tensor_copy(hstore[:, delta, :],
                                          pT[:].rearrange("p k b -> p (k b)"))

            # ================= loop 1: uni scan =================
            nc.vector.memset(hTs[:], 0.0)
            nc.vector.memset(st[:], 0.0)
            with tc.For_i(0, T, UNROLL, hint_engines=HINTS) as i:
                xt = iop.tile([128, UNROLL, 3, B], BF, tag="xt")
                nc.sync.dma_start(
                    xt[:], d["xT"][ds(i, UNROLL)].rearrange("t p k b -> p t k b"))
                hst = iop.tile([128, UNROLL, 4 * B], BF, tag="hst")
                zb = emit_proj(lambda k: xt[:, 0, k, :], 3, wxu, None)
                for dt in range(UNROLL):
                    emit_rec(zb, whu)
                    zb_cur, zb = zb, None
                    if dt + 1 < UNROLL:
                        zb = emit_proj(
                            lambda k, dt=dt: xt[:, dt + 1, k, :], 3, wxu, None)
                    step_tail(zb_cur, dt, hst, False)
                nc.sync.dma_start(
                    d["hT"][ds(i, UNROLL)].rearrange("t p e -> p t e"), hst[:])

            # ================= loop 2: dir scan =================
            nc.vector.memset(hTs[:], 0.0)
            nc.vector.memset(st[:], 0.0)
            nc.vector.memset(rmax[:], -1e30)
            with tc.For_i(0, T, UNROLL, hint_engines=HINTS) as i:
                ht1 = iop.tile([128, UNROLL, 4, B], BF, tag="ht1")
                for dt in range(UNROLL):
                    # fwd: t = i+dt ; bwd: t = (T-1) - (i+dt)
                    tt = i + dt
                    tsrc = nc.s_assert_within(
                        tt + s * (T - 1 - 2 * tt), 0, T - 1,
                        skip_runtime_assert=True)
                    nc.sync.dma_start(
                        ht1[:, dt, :, :].rearrange("p k b -> p (k b)"),
                        d["hT"][ds(tsrc, 1)].rearrange("t p e -> (t p) e"))
                bd_ = bdT if with_bias else None
                zb = emit_proj(lambda k: ht1[:, 0, k, :], 4, wxd, bd_)
                for dt in range(UNROLL):
                    emit_rec(zb, whd)
                    zb_cur, zb = zb, None
                    if dt + 1 < UNROLL:
                        zb = emit_proj(
                            lambda k, dt=dt: ht1[:, dt + 1, k, :], 4, wxd, bd_)
                    step_tail(zb_cur, dt, None, True)
            nc.sync.dma_start(d["rmax"], rmax[:])

    with tile.TileContext(nc) as tc:
        for _ in range(rep):
            build(tc)
    nc.compile()
    return nc


_GATE_PERM = np.r_[H:2 * H, 0:H, 3 * H:4 * H, 2 * H:3 * H]  # [i f g o]->[f i o g]


def _dup_cols(w):
    """[rows, 4H] gate-ordered [f i o g] -> [rows, 2j, (n,q',c)=HB] so that
    moving slice w2[:, j, n*512:(n+1)*512] covers gates (2n, 2n+1), j-half."""
    rows = w.shape[0]
    # w[r, q*H + j*256 + c] -> w2[r, j, q*256 + c]
    w5 = w.reshape(rows, 4, 2, 256)            # [r, q, j, c]
    return np.ascontiguousarray(w5.transpose(0, 2, 1, 3).reshape(rows, 2, HB))


def _prep_shared(x):
    """x [B,T,E] -> xT [T,128,3,64] bf16 with ones-row at E-index 300."""
    xpad = np.zeros((B, T, 384), np.float32)
    xpad[:, :, :E] = x
    xpad[:, :, E] = 1.0
    xT = xpad.transpose(1, 2, 0).reshape(T, 3, 128, B).transpose(0, 2, 1, 3)
    return np.ascontiguousarray(xT.astype(BF16))


def _chunk(w, nk):
    """[nk*128, 2, HB] -> [128, nk, 2, HB] bf16."""
    return np.ascontiguousarray(
        w.reshape(nk, 128, 2, HB).transpose(1, 0, 2, 3).astype(BF16))


def _prep_core(xT, wx_u, wh_u, b_u, wx_d, wh_d, b_d):
    wx_u = np.asarray(wx_u, np.float32)[:, _GATE_PERM]
    wh_u = np.asarray(wh_u, np.float32)[:, _GATE_PERM]
    b_u = np.asarray(b_u, np.float32)[_GATE_PERM]
    wx_d = np.asarray(wx_d, np.float32)[:, _GATE_PERM]
    wh_d = np.asarray(wh_d, np.float32)[:, _GATE_PERM]
    b_d = np.asarray(b_d, np.float32)[_GATE_PERM]
    wxu_pad = np.zeros((384, FOUR_H), np.float32)
    wxu_pad[:E] = wx_u
    wxu_pad[E] = b_u
    return {
        "xT": xT,
        "wxu": _chunk(_dup_cols(wxu_pad), 3),
        "whu": _chunk(_dup_cols(wh_u), 4),
        "wxd": _chunk(_dup_cols(wx_d), 4),
        "whd": _chunk(_dup_cols(wh_d), 4),
        "bdT": np.ascontiguousarray(_dup_cols(b_d[None, :]).astype(BF16)),
        "one1": np.ones((1, B), BF16),
        "id64": np.eye(B, dtype=np.float32),
    }


def _run(in_maps, rep=1, with_bias=True):
    from concourse.bass_utils import run_bass_kernel_spmd
    key = f"nc{rep}_{with_bias}"
    if key not in _CACHE:
        _CACHE[key] = _build_program(rep, with_bias)
    return run_bass_kernel_spmd(_CACHE[key], in_maps, core_ids=list(range(6)))


def build_in_maps(x, uni_Wx, uni_Wh, uni_b, fwd_Wx, fwd_Wh, fwd_b,
                  bwd_Wx, bwd_Wh, bwd_b):
    xT = _prep_shared(np.asarray(x, np.float32))
    in_maps = []
    for c in range(6):
        br = c % 3
        if c < 3:
            wx_d, wh_d, b_d = fwd_Wx[br], fwd_Wh[br], fwd_b[br]
        else:
            wx_d, wh_d, b_d = bwd_Wx[br], bwd_Wh[br], bwd_b[br]
        in_maps.append(_prep_core(xT, uni_Wx[br], uni_Wh[br], uni_b[br],
                                  wx_d, wh_d, b_d))
    return in_maps


def kernel(x, uni_Wx, uni_Wh, uni_b, fwd_Wx, fwd_Wh, fwd_b,
           bwd_Wx, bwd_Wh, bwd_b):
    in_maps = build_in_maps(x, uni_Wx, uni_Wh, uni_b, fwd_Wx, fwd_Wh, fwd_b,
                            bwd_Wx, bwd_Wh, bwd_b)
    wb = bool(np.any(np.asarray(fwd_b)) or np.any(np.asarray(bwd_b)))
    res = _run(in_maps, with_bias=wb)
    out = np.empty((B, NB * 2 * H), np.float32)
    for c in range(6):
        br = c % 3
        off = br * 2 * H + (0 if c < 3 else H)
        out[:, off:off + H] = res.results[c]["rmax"]
    return out
